# revision 25
# baseline (speedup 1.0000x reference)
"""GQA attention block (QKV proj + RoPE + attention + out proj) on 8 TRN2 cores.

Sharding: tensor-parallel over heads. Each core gets 4 Q heads + their single
shared KV head (GQA groups intact), plus the matching Wo row-slice. Cores
produce partial [B*S, D] outputs that the host sums.

Per-core dataflow (all matmuls bf16, fp32 PSUM accumulate):
  - host pre-transposes x -> xT [B, D, S] so projections run as W.T @ x.T
    with head-dims on partitions.
  - Q proj per head-pair: psum[128, 512] = sum_kt Wq[kt,128].T @ xT[kt,512];
    bias fused into the ACT psum->sbuf copy; RoPE (split-half layout, host
    permutes Wq/Wk columns so rotation halves are contiguous rows) on DVE,
    applied once per (b, pair) over the full sequence.
  - K+V packed in one projection (K rows 0-63, V rows 64-127).
  - scoresT[t,s] for a head pair land in ONE 2-bank psum tile [128, 1024]
    via row-packed K=64 matmuls (head A rows 0-63 -> cols 0:512, head B rows
    64-127 -> cols 512:1024); ONE exp [128, 1024] per t-tile amortizes the
    ScalarE per-instruction overhead.
  - AV: lhsT = [v | ones x 64] so psum rows 0-63 accumulate o.T and rows
    64-127 accumulate the softmax denominator (replicated); normalize with
    reciprocal_approx_fast (base-0 only!) + multiply on the way to SBUF.
  - O proj: psum[s,e] = sum oT[128,s].T @ Wo[128,e]; copy to SBUF; DMA out.
"""

import sys

sys.path.insert(0, "/opt/trn_rl_repo")

from contextlib import ExitStack

import numpy as np
import ml_dtypes

import concourse.bass as bass  # noqa: F401
import concourse.tile as tile
from concourse import bacc, mybir
from concourse.bass_utils import run_bass_kernel_spmd
from concourse.masks import make_identity

BF16 = mybir.dt.bfloat16
F32 = mybir.dt.float32
F16 = mybir.dt.float16
AF = mybir.ActivationFunctionType

B, S, D = 2, 2048, 2048
QH, KVH, HD = 32, 8, 64
NCORES = 8
QH_LOC = QH // NCORES  # 4 q-heads per core
P = 128
SS = 512  # s-slice (psum free dim)
NSS = S // SS  # 4
KT = D // P  # 16 contraction tiles for projections
NT = S // P  # 16 t-tiles for attention
NPAIR = QH_LOC // 2  # 2 head-pairs per core
SCALE = 1.0 / float(np.sqrt(HD))

# within-head dim permutation: even dims (cos half) first, odd dims second
_PERM = np.concatenate([np.arange(0, HD, 2), np.arange(1, HD, 2)])

DEBUG_DUMPS = False


def _rope(nc, tmp_pool, qsl, cos_sb, sin_sb, head_bases, cols):
    """In-place RoPE on qsl rows [hb, hb+64) for each hb (split-half layout).

    qsl covers sequence columns `cols` (a slice); the tables are indexed with
    the same columns. Both SBUF inputs of each tensor_tensor op must share a
    base partition (walrus verifier); tables are 32-row periodic so any
    aligned row block works.
    """
    width = cols.stop - cols.start
    t1 = tmp_pool.tile([P, width], BF16, tag="ropetmp1")
    t2 = tmp_pool.tile([P, width], BF16, tag="ropetmp2")
    for hb in head_bases:
        lo = slice(hb, hb + 32)
        hi = slice(hb + 32, hb + 64)
        x0 = qsl[lo]
        x1 = qsl[hi]
        nc.vector.tensor_mul(t1[lo], x0, cos_sb[lo, cols])  # x0*cos @ lo
        nc.vector.tensor_mul(t2[lo], x1, sin_sb[hi, cols])  # x1*sin -> lo
        nc.vector.tensor_mul(t1[hi], x0, sin_sb[lo, cols])  # x0*sin -> hi
        nc.vector.tensor_mul(t2[hi], x1, cos_sb[hi, cols])  # x1*cos @ hi
        nc.vector.tensor_sub(x0, t1[lo], t2[lo])
        nc.vector.tensor_add(x1, t1[hi], t2[hi])


def build_nc():
    nc = bacc.Bacc("TRN2", target_bir_lowering=False, debug=False, num_devices=NCORES)

    xt_d = nc.dram_tensor("xt", [B, NSS, P, KT, SS], BF16, kind="ExternalInput")
    wq_d = nc.dram_tensor("wq", [P, KT, NPAIR * P], BF16, kind="ExternalInput")
    wkv_d = nc.dram_tensor("wkv", [P, KT, P], BF16, kind="ExternalInput")
    wo_d = nc.dram_tensor("wo", [P, 2, D], BF16, kind="ExternalInput")
    cos_d = nc.dram_tensor("cost", [P, S], BF16, kind="ExternalInput")
    sin_d = nc.dram_tensor("sint", [P, S], BF16, kind="ExternalInput")
    bq_d = nc.dram_tensor("bq", [P, NPAIR], F32, kind="ExternalInput")
    bkv_d = nc.dram_tensor("bkv", [P, 1], F32, kind="ExternalInput")
    out_d = nc.dram_tensor("out", [B * S, D], F16, kind="ExternalOutput")
    if DEBUG_DUMPS:
        dqa_d = nc.dram_tensor("dqa", [P, B, NPAIR, S], BF16, kind="ExternalOutput")
        dkv_d = nc.dram_tensor("dkv", [P, B, S], BF16, kind="ExternalOutput")
        dvaug_d = nc.dram_tensor("dvaug", [P, B, NT, P], BF16, kind="ExternalOutput")
        dot_d = nc.dram_tensor("dot", [P, B, 2, S], BF16, kind="ExternalOutput")

    with tile.TileContext(nc) as tc:
        with ExitStack() as ctx:
            consts = ctx.enter_context(tc.tile_pool(name="consts", bufs=1))
            acts = ctx.enter_context(tc.tile_pool(name="acts", bufs=1))
            xpool = ctx.enter_context(tc.tile_pool(name="xt", bufs=4))
            tmp_pool = ctx.enter_context(tc.tile_pool(name="tmp", bufs=2))
            ppool = ctx.enter_context(tc.tile_pool(name="pexp", bufs=6))
            rpool = ctx.enter_context(tc.tile_pool(name="recip", bufs=2))
            opool = ctx.enter_context(tc.tile_pool(name="osb", bufs=3))
            # PSUM: scores 2x[128,1024] (4 banks) + av 2x[128,512] (2) +
            # proj/fin shared 2x[128,512] (2) = 8 banks exactly.
            sc_ps = ctx.enter_context(tc.tile_pool(name="sc", bufs=2, space="PSUM"))
            av_ps = ctx.enter_context(tc.tile_pool(name="av", bufs=2, space="PSUM"))
            pf_ps = ctx.enter_context(tc.tile_pool(name="pf", bufs=2, space="PSUM"))

            # ---- resident constants ----
            wq_sb = consts.tile([P, KT, NPAIR * P], BF16)
            nc.sync.dma_start(wq_sb[:], wq_d.ap())
            wkv_sb = consts.tile([P, KT, P], BF16)
            nc.sync.dma_start(wkv_sb[:], wkv_d.ap())
            wo_sb = consts.tile([P, 2, D], BF16)
            nc.sync.dma_start(wo_sb[:], wo_d.ap())
            cos_sb = consts.tile([P, S], BF16)
            nc.sync.dma_start(cos_sb[:], cos_d.ap())
            sin_sb = consts.tile([P, S], BF16)
            nc.sync.dma_start(sin_sb[:], sin_d.ap())
            bq_sb = consts.tile([P, NPAIR], F32)
            nc.sync.dma_start(bq_sb[:], bq_d.ap())
            bkv_sb = consts.tile([P, 1], F32)
            nc.sync.dma_start(bkv_sb[:], bkv_d.ap())
            ident = consts.tile([P, P], BF16)
            make_identity(nc, ident[:])

            # ---- persistent activations ----
            qa_sb = acts.tile([P, B, NPAIR, S], BF16)  # rotated q, pair tiles
            kv_sb = acts.tile([P, B, S], BF16)  # rows 0-63 k(rot), 64-127 v
            kk_sb = acts.tile([P, B, S], BF16)  # rows 64-127 = copy of k
            vaug_sb = acts.tile([P, B, NT, P], BF16)  # [t, 0:64]=v, [64:128]=1
            ot_sb = acts.tile([P, B, 2, S], BF16)  # normalized o.T stacked

            nc.any.memset(vaug_sb[:, :, :, HD:], 1.0)

            def proj_phase(b):
                """Joint KV+Q projections for batch b from one xT pass, with
                per-half RoPE, k-duplication and V-transposes inline."""
                for ss in range(NSS):
                    sl = slice(ss * SS, (ss + 1) * SS)
                    xt_t = xpool.tile([P, KT, SS], BF16, tag="xt")
                    nc.sync.dma_start(xt_t[:], xt_d.ap()[b, ss])
                    # K+V packed projection
                    ps = pf_ps.tile([P, SS], F32, tag="pf")
                    for kt in range(KT):
                        nc.tensor.matmul(
                            ps[:],
                            wkv_sb[:, kt],
                            xt_t[:, kt],
                            start=(kt == 0),
                            stop=(kt == KT - 1),
                        )
                    nc.scalar.activation(
                        kv_sb[:, b, sl], ps[:], AF.Identity, bias=bkv_sb[:]
                    )
                    # Q projections
                    for pair in range(NPAIR):
                        ps = pf_ps.tile([P, SS], F32, tag="pf")
                        for kt in range(KT):
                            nc.tensor.matmul(
                                ps[:],
                                wq_sb[:, kt, pair * P : (pair + 1) * P],
                                xt_t[:, kt],
                                start=(kt == 0),
                                stop=(kt == KT - 1),
                            )
                        nc.scalar.activation(
                            qa_sb[:, b, pair, sl],
                            ps[:],
                            AF.Identity,
                            bias=bq_sb[:, pair : pair + 1],
                        )
                    if ss % 2 == 1:
                        hl = slice((ss - 1) * SS, (ss + 1) * SS)
                        _rope(nc, tmp_pool, kv_sb[:, b, hl], cos_sb, sin_sb, (0,), hl)
                        nc.vector.tensor_copy(kk_sb[HD:P, b, hl], kv_sb[0:HD, b, hl])
                        for pair in range(NPAIR):
                            _rope(
                                nc,
                                tmp_pool,
                                qa_sb[:, b, pair, hl],
                                cos_sb,
                                sin_sb,
                                (0, HD),
                                hl,
                            )
                        for ci in range((ss - 1) * (SS // P), (ss + 1) * (SS // P)):
                            csl = slice(ci * P, (ci + 1) * P)
                            # v rows 64-127 of the kv chunk -> [t, d] layout,
                            # via XBAR DMA transpose (no PE / PSUM involved)
                            nc.sync.dma_start_transpose(
                                vaug_sb[:, b, ci, 0:HD], kv_sb[HD:P, b, csl]
                            )

            def attn_phase(b, tail=None):
                for pair in range(NPAIR):
                    for ss in range(NSS):
                        sl = slice(ss * SS, (ss + 1) * SS)
                        po0 = av_ps.tile([P, SS], F32, tag="av")
                        po1 = av_ps.tile([P, SS], F32, tag="av")
                        for tt in range(NT):
                            csl = slice(tt * P, (tt + 1) * P)
                            # both heads' scoresT in one 2-bank psum tile
                            sc = sc_ps.tile([P, 2 * SS], F32, tag="sc")
                            nc.tensor.matmul(
                                sc[:, 0:SS],
                                kv_sb[0:HD, b, csl],
                                qa_sb[0:HD, b, pair, sl],
                                start=True,
                                stop=True,
                            )
                            nc.tensor.matmul(
                                sc[:, SS : 2 * SS],
                                kk_sb[HD:P, b, csl],
                                qa_sb[HD:P, b, pair, sl],
                                start=True,
                                stop=True,
                                tile_position=(HD, 0),
                            )
                            pa = ppool.tile([P, 2 * SS], BF16, tag="p")
                            nc.scalar.activation(pa[:], sc[:], AF.Exp, scale=SCALE)
                            nc.tensor.matmul(
                                po0[:],
                                vaug_sb[:, b, tt],
                                pa[:, 0:SS],
                                start=(tt == 0),
                                stop=(tt == NT - 1),
                            )
                            nc.tensor.matmul(
                                po1[:],
                                vaug_sb[:, b, tt],
                                pa[:, SS : 2 * SS],
                                start=(tt == 0),
                                stop=(tt == NT - 1),
                            )
                        # normalize: rows 64-127 hold sumexp (replicated).
                        # reciprocal_approx_fast mishandles nonzero partition
                        # bases, so copy the band down to base 0 first.
                        r0 = rpool.tile([HD, SS], F32, tag="r")
                        r1 = rpool.tile([HD, SS], F32, tag="r")
                        se0 = rpool.tile([HD, SS], F32, tag="se")
                        se1 = rpool.tile([HD, SS], F32, tag="se")
                        nc.vector.tensor_copy(se0[:], po0[HD:P])
                        nc.vector.tensor_copy(se1[:], po1[HD:P])
                        nc.vector.reciprocal_approx_fast(r0[:], se0[:])
                        nc.vector.reciprocal_approx_fast(r1[:], se1[:])
                        nc.vector.tensor_mul(
                            ot_sb[0:HD, b, pair, sl], po0[0:HD], r0[:]
                        )
                        nc.vector.tensor_mul(
                            ot_sb[HD:P, b, pair, sl], po1[0:HD], r1[:]
                        )
                        if pair == NPAIR - 1 and tail is not None:
                            tail(ss)

            def oproj_ss(b, ss):
                for sc_i in range(ss * (SS // P), (ss + 1) * (SS // P)):
                    scl = slice(sc_i * P, (sc_i + 1) * P)
                    for es in range(NSS):
                        esl = slice(es * SS, (es + 1) * SS)
                        pf = pf_ps.tile([P, SS], F32, tag="pf")
                        for kt2 in range(2):
                            nc.tensor.matmul(
                                pf[:],
                                ot_sb[:, b, kt2, scl],
                                wo_sb[:, kt2, esl],
                                start=(kt2 == 0),
                                stop=(kt2 == 1),
                            )
                        ob = opool.tile([P, SS], F16, tag="osb")
                        nc.vector.tensor_copy(ob[:], pf[:])
                        nc.sync.dma_start(
                            out_d.ap()[b * S + sc_i * P : b * S + (sc_i + 1) * P, esl],
                            ob[:],
                        )

            # batch-interleaved phase order: b1's projections are emitted
            # before b0's output projection so they fill PE gaps during b0's
            # (ScalarE-bound) attention phase.
            proj_phase(0)
            attn_phase(0, tail=lambda ss: oproj_ss(0, ss))
            proj_phase(1)
            attn_phase(1, tail=lambda ss: oproj_ss(1, ss))

            if DEBUG_DUMPS:
                nc.sync.dma_start(dqa_d.ap(), qa_sb[:])
                nc.sync.dma_start(dkv_d.ap(), kv_sb[:])
                nc.sync.dma_start(dvaug_d.ap(), vaug_sb[:])
                nc.sync.dma_start(dot_d.ap(), ot_sb[:])

    nc.compile()
    return nc


_NC_CACHE = None


def _get_nc():
    global _NC_CACHE
    if _NC_CACHE is None:
        _NC_CACHE = build_nc()
    return _NC_CACHE


def prepare_in_maps(x, freqs, Wq, bq, Wk, bk, Wv, bv, Wo, bo):
    x = np.asarray(x, np.float32)
    freqs = np.asarray(freqs, np.float32)
    Wq = np.asarray(Wq, np.float32)
    bq = np.asarray(bq, np.float32)
    Wk = np.asarray(Wk, np.float32)
    bk = np.asarray(bk, np.float32)
    Wv = np.asarray(Wv, np.float32)
    bv = np.asarray(bv, np.float32)
    Wo = np.asarray(Wo, np.float32)

    bf = ml_dtypes.bfloat16
    # [B, S, D] -> [B, D, S] -> tiled [B, NSS, P(p), KT(o), SS] with
    # d = o*P + p and s = ss*SS + j, so each (b, ss) DMA is contiguous.
    xt = (
        x.transpose(0, 2, 1)
        .reshape(B, KT, P, NSS, SS)
        .transpose(0, 3, 2, 1, 4)
    )
    xt = np.ascontiguousarray(xt).astype(bf)
    cost = np.ascontiguousarray(np.tile(freqs[:, :, 0].T, (4, 1))).astype(bf)
    sint = np.ascontiguousarray(np.tile(freqs[:, :, 1].T, (4, 1))).astype(bf)

    in_maps = []
    for c in range(NCORES):
        hq = slice(c * QH_LOC * HD, (c + 1) * QH_LOC * HD)
        hk = slice(c * HD, (c + 1) * HD)
        wq_c = Wq[:, hq].reshape(D, QH_LOC, HD)[:, :, _PERM].reshape(D, QH_LOC * HD)
        bq_c = bq[hq].reshape(QH_LOC, HD)[:, _PERM].reshape(NPAIR, P).T
        wk_c = Wk[:, hk][:, _PERM]
        wv_c = Wv[:, hk]
        wkv_c = np.concatenate([wk_c, wv_c], axis=1)
        bkv_c = np.concatenate([bk[hk][_PERM], bv[hk]])[:, None]
        wo_c = Wo[hq, :]
        in_maps.append(
            {
                "xt": xt,
                "wq": np.ascontiguousarray(
                    wq_c.reshape(KT, P, NPAIR * P).transpose(1, 0, 2)
                ).astype(bf),
                "wkv": np.ascontiguousarray(
                    wkv_c.reshape(KT, P, P).transpose(1, 0, 2)
                ).astype(bf),
                "wo": np.ascontiguousarray(
                    wo_c.reshape(2, P, D).transpose(1, 0, 2)
                ).astype(bf),
                "cost": cost,
                "sint": sint,
                "bq": np.ascontiguousarray(bq_c, dtype=np.float32),
                "bkv": np.ascontiguousarray(bkv_c, dtype=np.float32),
            }
        )
    return in_maps


def run(in_maps, trace=False, **kw):
    nc = _get_nc()
    return run_bass_kernel_spmd(nc, in_maps, list(range(NCORES)), trace=trace, **kw)


def kernel(**inputs):
    in_maps = prepare_in_maps(**{k: inputs[k] for k in (
        "x", "freqs", "Wq", "bq", "Wk", "bk", "Wv", "bv", "Wo", "bo")})
    res = run(in_maps, trace=False)
    acc = np.zeros((B * S, D), np.float64)
    for r in res.results:
        acc += r["out"].astype(np.float64)
    out = acc.astype(np.float32) + np.asarray(inputs["bo"], np.float32)[None, :]
    return out.reshape(B, S, D)


# revision 26
# speedup vs baseline: 1.3198x; 1.3198x over previous
"""GQA attention block (QKV proj + RoPE + attention + out proj) on 8 TRN2 cores.

Sharding: tensor-parallel over heads. Each core gets 4 Q heads + their single
shared KV head (GQA groups intact), plus the matching Wo row-slice. Cores
produce partial [B*S, D] outputs that the host sums.

Per-core dataflow (all matmuls bf16, fp32 PSUM accumulate):
  - host pre-transposes x -> xT [B, D, S] so projections run as W.T @ x.T
    with head-dims on partitions.
  - Q proj per head-pair: psum[128, 512] = sum_kt Wq[kt,128].T @ xT[kt,512];
    bias fused into the ACT psum->sbuf copy; RoPE (split-half layout, host
    permutes Wq/Wk columns so rotation halves are contiguous rows) on DVE,
    applied once per (b, pair) over the full sequence.
  - K+V packed in one projection (K rows 0-63, V rows 64-127).
  - scoresT[t,s] for a head pair land in ONE 2-bank psum tile [128, 1024]
    via row-packed K=64 matmuls (head A rows 0-63 -> cols 0:512, head B rows
    64-127 -> cols 512:1024); ONE exp [128, 1024] per t-tile amortizes the
    ScalarE per-instruction overhead.
  - AV: lhsT = [v | ones x 64] so psum rows 0-63 accumulate o.T and rows
    64-127 accumulate the softmax denominator (replicated); normalize with
    reciprocal_approx_fast (base-0 only!) + multiply on the way to SBUF.
  - O proj: psum[s,e] = sum oT[128,s].T @ Wo[128,e]; copy to SBUF; DMA out.
"""

import sys

sys.path.insert(0, "/opt/trn_rl_repo")

from contextlib import ExitStack

import numpy as np
import ml_dtypes

import concourse.bass as bass  # noqa: F401
import concourse.tile as tile
from concourse import bacc, mybir
from concourse.bass_utils import run_bass_kernel_spmd
from concourse.masks import make_identity

BF16 = mybir.dt.bfloat16
F32 = mybir.dt.float32
F16 = mybir.dt.float16
AF = mybir.ActivationFunctionType

B, S, D = 2, 2048, 2048
QH, KVH, HD = 32, 8, 64
NCORES = 8
QH_LOC = QH // NCORES  # 4 q-heads per core
P = 128
SS = 512  # s-slice (psum free dim)
NSS = S // SS  # 4
KT = D // P  # 16 contraction tiles for projections
NT = S // P  # 16 t-tiles for attention
NPAIR = QH_LOC // 2  # 2 head-pairs per core
SCALE = 1.0 / float(np.sqrt(HD))

# within-head dim permutation: even dims (cos half) first, odd dims second
_PERM = np.concatenate([np.arange(0, HD, 2), np.arange(1, HD, 2)])

DEBUG_DUMPS = False


def _rope(nc, tmp_pool, qsl, cos_sb, sin_sb, head_bases, cols):
    """In-place RoPE on qsl rows [hb, hb+64) for each hb (split-half layout).

    qsl covers sequence columns `cols` (a slice); the tables are indexed with
    the same columns. Both SBUF inputs of each tensor_tensor op must share a
    base partition (walrus verifier); tables are 32-row periodic so any
    aligned row block works.
    """
    width = cols.stop - cols.start
    t1 = tmp_pool.tile([P, width], BF16, tag="ropetmp1")
    t2 = tmp_pool.tile([P, width], BF16, tag="ropetmp2")
    for hb in head_bases:
        lo = slice(hb, hb + 32)
        hi = slice(hb + 32, hb + 64)
        x0 = qsl[lo]
        x1 = qsl[hi]
        nc.vector.tensor_mul(t1[lo], x0, cos_sb[lo, cols])  # x0*cos @ lo
        nc.vector.tensor_mul(t2[lo], x1, sin_sb[hi, cols])  # x1*sin -> lo
        nc.vector.tensor_mul(t1[hi], x0, sin_sb[lo, cols])  # x0*sin -> hi
        nc.vector.tensor_mul(t2[hi], x1, cos_sb[hi, cols])  # x1*cos @ hi
        nc.vector.tensor_sub(x0, t1[lo], t2[lo])
        nc.vector.tensor_add(x1, t1[hi], t2[hi])


def build_nc():
    nc = bacc.Bacc("TRN2", target_bir_lowering=False, debug=False, num_devices=NCORES)

    xt_d = nc.dram_tensor("xt", [B, NSS, P, KT, SS], BF16, kind="ExternalInput")
    wq_d = nc.dram_tensor("wq", [P, KT, NPAIR * P], BF16, kind="ExternalInput")
    wkv_d = nc.dram_tensor("wkv", [P, KT, P], BF16, kind="ExternalInput")
    wo_d = nc.dram_tensor("wo", [P, 2, D], BF16, kind="ExternalInput")
    cos_d = nc.dram_tensor("cost", [P, S], BF16, kind="ExternalInput")
    sin_d = nc.dram_tensor("sint", [P, S], BF16, kind="ExternalInput")
    bq_d = nc.dram_tensor("bq", [P, NPAIR], F32, kind="ExternalInput")
    bkv_d = nc.dram_tensor("bkv", [P, 1], F32, kind="ExternalInput")
    out_d = nc.dram_tensor("out", [B * S, D], F16, kind="ExternalOutput")
    if DEBUG_DUMPS:
        dqa_d = nc.dram_tensor("dqa", [P, B, NPAIR, S], BF16, kind="ExternalOutput")
        dkv_d = nc.dram_tensor("dkv", [P, B, S], BF16, kind="ExternalOutput")
        dvaug_d = nc.dram_tensor("dvaug", [P, B, NT, P], BF16, kind="ExternalOutput")
        dot_d = nc.dram_tensor("dot", [P, B, 2, S], BF16, kind="ExternalOutput")

    with tile.TileContext(nc) as tc:
        with ExitStack() as ctx:
            consts = ctx.enter_context(tc.tile_pool(name="consts", bufs=1))
            acts = ctx.enter_context(tc.tile_pool(name="acts", bufs=1))
            xpool = ctx.enter_context(tc.tile_pool(name="xt", bufs=4))
            tmp_pool = ctx.enter_context(tc.tile_pool(name="tmp", bufs=2))
            ppool = ctx.enter_context(tc.tile_pool(name="pexp", bufs=6))
            rpool = ctx.enter_context(tc.tile_pool(name="recip", bufs=2))
            opool = ctx.enter_context(tc.tile_pool(name="osb", bufs=3))
            # PSUM: scores 2x[128,1024] (4 banks) + av 2x[128,512] (2) +
            # proj/fin shared 2x[128,512] (2) = 8 banks exactly.
            sc_ps = ctx.enter_context(tc.tile_pool(name="sc", bufs=2, space="PSUM"))
            av_ps = ctx.enter_context(tc.tile_pool(name="av", bufs=2, space="PSUM"))
            pf_ps = ctx.enter_context(tc.tile_pool(name="pf", bufs=2, space="PSUM"))

            # ---- resident constants ----
            wq_sb = consts.tile([P, KT, NPAIR * P], BF16)
            nc.sync.dma_start(wq_sb[:], wq_d.ap())
            wkv_sb = consts.tile([P, KT, P], BF16)
            nc.sync.dma_start(wkv_sb[:], wkv_d.ap())
            wo_sb = consts.tile([P, 2, D], BF16)
            nc.sync.dma_start(wo_sb[:], wo_d.ap())
            cos_sb = consts.tile([P, S], BF16)
            nc.sync.dma_start(cos_sb[:], cos_d.ap())
            sin_sb = consts.tile([P, S], BF16)
            nc.sync.dma_start(sin_sb[:], sin_d.ap())
            bq_sb = consts.tile([P, NPAIR], F32)
            nc.sync.dma_start(bq_sb[:], bq_d.ap())
            bkv_sb = consts.tile([P, 1], F32)
            nc.sync.dma_start(bkv_sb[:], bkv_d.ap())
            ident = consts.tile([P, P], BF16)
            make_identity(nc, ident[:])

            # ---- persistent activations ----
            qa_sb = acts.tile([P, B, NPAIR, S], BF16)  # rotated q, pair tiles
            kv_sb = acts.tile([P, B, S], BF16)  # rows 0-63 k(rot), 64-127 v
            kk_sb = acts.tile([P, B, S], BF16)  # rows 64-127 = copy of k
            vaug_sb = acts.tile([P, B, NT, P], BF16)  # [t, 0:64]=v, [64:128]=1
            ot_sb = acts.tile([P, B, 2, S], BF16)  # normalized o.T stacked

            nc.any.memset(vaug_sb[:, :, :, HD:], 1.0)

            def proj_phase(b):
                """Joint KV+Q projections for batch b from one xT pass, with
                per-half RoPE, k-duplication and V-transposes inline."""
                for ss in range(NSS):
                    sl = slice(ss * SS, (ss + 1) * SS)
                    xt_t = xpool.tile([P, KT, SS], BF16, tag="xt")
                    nc.sync.dma_start(xt_t[:], xt_d.ap()[b, ss])
                    # K+V packed projection
                    ps = pf_ps.tile([P, SS], F32, tag="pf")
                    for kt in range(KT):
                        nc.tensor.matmul(
                            ps[:],
                            wkv_sb[:, kt],
                            xt_t[:, kt],
                            start=(kt == 0),
                            stop=(kt == KT - 1),
                        )
                    nc.scalar.activation(
                        kv_sb[:, b, sl], ps[:], AF.Identity, bias=bkv_sb[:]
                    )
                    # Q projections
                    for pair in range(NPAIR):
                        ps = pf_ps.tile([P, SS], F32, tag="pf")
                        for kt in range(KT):
                            nc.tensor.matmul(
                                ps[:],
                                wq_sb[:, kt, pair * P : (pair + 1) * P],
                                xt_t[:, kt],
                                start=(kt == 0),
                                stop=(kt == KT - 1),
                            )
                        nc.scalar.activation(
                            qa_sb[:, b, pair, sl],
                            ps[:],
                            AF.Identity,
                            bias=bq_sb[:, pair : pair + 1],
                        )
                    if ss % 2 == 1:
                        hl = slice((ss - 1) * SS, (ss + 1) * SS)
                        _rope(nc, tmp_pool, kv_sb[:, b, hl], cos_sb, sin_sb, (0,), hl)
                        nc.vector.tensor_copy(kk_sb[HD:P, b, hl], kv_sb[0:HD, b, hl])
                        for pair in range(NPAIR):
                            _rope(
                                nc,
                                tmp_pool,
                                qa_sb[:, b, pair, hl],
                                cos_sb,
                                sin_sb,
                                (0, HD),
                                hl,
                            )
                        for ci in range((ss - 1) * (SS // P), (ss + 1) * (SS // P)):
                            csl = slice(ci * P, (ci + 1) * P)
                            # v rows 64-127 of the kv chunk -> [t, d] layout,
                            # via XBAR DMA transpose (no PE / PSUM involved)
                            nc.sync.dma_start_transpose(
                                vaug_sb[:, b, ci, 0:HD], kv_sb[HD:P, b, csl]
                            )

            def attn_phase(b, tail=None):
                for pair in range(NPAIR):
                    for ss in range(NSS):
                        sl = slice(ss * SS, (ss + 1) * SS)
                        po0 = av_ps.tile([P, SS], F32, tag="av")
                        po1 = av_ps.tile([P, SS], F32, tag="av")
                        for tt in range(NT):
                            csl = slice(tt * P, (tt + 1) * P)
                            # both heads' scoresT in one 2-bank psum tile
                            sc = sc_ps.tile([P, 2 * SS], F32, tag="sc")
                            nc.tensor.matmul(
                                sc[:, 0:SS],
                                kv_sb[0:HD, b, csl],
                                qa_sb[0:HD, b, pair, sl],
                                start=True,
                                stop=True,
                            )
                            nc.tensor.matmul(
                                sc[:, SS : 2 * SS],
                                kk_sb[HD:P, b, csl],
                                qa_sb[HD:P, b, pair, sl],
                                start=True,
                                stop=True,
                                tile_position=(HD, 0),
                            )
                            pa = ppool.tile([P, 2 * SS], BF16, tag="p")
                            nc.scalar.activation(pa[:], sc[:], AF.Exp, scale=SCALE)
                            nc.tensor.matmul(
                                po0[:],
                                vaug_sb[:, b, tt],
                                pa[:, 0:SS],
                                start=(tt == 0),
                                stop=(tt == NT - 1),
                            )
                            nc.tensor.matmul(
                                po1[:],
                                vaug_sb[:, b, tt],
                                pa[:, SS : 2 * SS],
                                start=(tt == 0),
                                stop=(tt == NT - 1),
                            )
                        # normalize: rows 64-127 hold sumexp (replicated).
                        # reciprocal_approx_fast mishandles nonzero partition
                        # bases, so copy the band down to base 0 first.
                        r0 = rpool.tile([HD, SS], F32, tag="r")
                        r1 = rpool.tile([HD, SS], F32, tag="r")
                        se0 = rpool.tile([HD, SS], F32, tag="se")
                        se1 = rpool.tile([HD, SS], F32, tag="se")
                        nc.vector.tensor_copy(se0[:], po0[HD:P])
                        nc.vector.tensor_copy(se1[:], po1[HD:P])
                        nc.vector.reciprocal_approx_fast(r0[:], se0[:])
                        nc.vector.reciprocal_approx_fast(r1[:], se1[:])
                        nc.vector.tensor_mul(
                            ot_sb[0:HD, b, pair, sl], po0[0:HD], r0[:]
                        )
                        nc.vector.tensor_mul(
                            ot_sb[HD:P, b, pair, sl], po1[0:HD], r1[:]
                        )
                        if pair == NPAIR - 1 and tail is not None:
                            tail(ss)

            def oproj_ss(b, ss):
                for sc_i in range(ss * (SS // P), (ss + 1) * (SS // P)):
                    scl = slice(sc_i * P, (sc_i + 1) * P)
                    for es in range(NSS):
                        esl = slice(es * SS, (es + 1) * SS)
                        pf = pf_ps.tile([P, SS], F32, tag="pf")
                        for kt2 in range(2):
                            nc.tensor.matmul(
                                pf[:],
                                ot_sb[:, b, kt2, scl],
                                wo_sb[:, kt2, esl],
                                start=(kt2 == 0),
                                stop=(kt2 == 1),
                            )
                        ob = opool.tile([P, SS], F16, tag="osb")
                        nc.vector.tensor_copy(ob[:], pf[:])
                        nc.sync.dma_start(
                            out_d.ap()[b * S + sc_i * P : b * S + (sc_i + 1) * P, esl],
                            ob[:],
                        )

            # batch-interleaved phase order: b1's projections are emitted
            # before b0's output projection so they fill PE gaps during b0's
            # (ScalarE-bound) attention phase.
            proj_phase(0)
            attn_phase(0)
            proj_phase(1)
            for _ss in range(NSS):
                oproj_ss(0, _ss)
            attn_phase(1, tail=lambda ss: oproj_ss(1, ss))

            if DEBUG_DUMPS:
                nc.sync.dma_start(dqa_d.ap(), qa_sb[:])
                nc.sync.dma_start(dkv_d.ap(), kv_sb[:])
                nc.sync.dma_start(dvaug_d.ap(), vaug_sb[:])
                nc.sync.dma_start(dot_d.ap(), ot_sb[:])

    nc.compile()
    return nc


_NC_CACHE = None


def _get_nc():
    global _NC_CACHE
    if _NC_CACHE is None:
        _NC_CACHE = build_nc()
    return _NC_CACHE


def prepare_in_maps(x, freqs, Wq, bq, Wk, bk, Wv, bv, Wo, bo):
    x = np.asarray(x, np.float32)
    freqs = np.asarray(freqs, np.float32)
    Wq = np.asarray(Wq, np.float32)
    bq = np.asarray(bq, np.float32)
    Wk = np.asarray(Wk, np.float32)
    bk = np.asarray(bk, np.float32)
    Wv = np.asarray(Wv, np.float32)
    bv = np.asarray(bv, np.float32)
    Wo = np.asarray(Wo, np.float32)

    bf = ml_dtypes.bfloat16
    # [B, S, D] -> [B, D, S] -> tiled [B, NSS, P(p), KT(o), SS] with
    # d = o*P + p and s = ss*SS + j, so each (b, ss) DMA is contiguous.
    xt = (
        x.transpose(0, 2, 1)
        .reshape(B, KT, P, NSS, SS)
        .transpose(0, 3, 2, 1, 4)
    )
    xt = np.ascontiguousarray(xt).astype(bf)
    cost = np.ascontiguousarray(np.tile(freqs[:, :, 0].T, (4, 1))).astype(bf)
    sint = np.ascontiguousarray(np.tile(freqs[:, :, 1].T, (4, 1))).astype(bf)

    in_maps = []
    for c in range(NCORES):
        hq = slice(c * QH_LOC * HD, (c + 1) * QH_LOC * HD)
        hk = slice(c * HD, (c + 1) * HD)
        wq_c = Wq[:, hq].reshape(D, QH_LOC, HD)[:, :, _PERM].reshape(D, QH_LOC * HD)
        bq_c = bq[hq].reshape(QH_LOC, HD)[:, _PERM].reshape(NPAIR, P).T
        wk_c = Wk[:, hk][:, _PERM]
        wv_c = Wv[:, hk]
        wkv_c = np.concatenate([wk_c, wv_c], axis=1)
        bkv_c = np.concatenate([bk[hk][_PERM], bv[hk]])[:, None]
        wo_c = Wo[hq, :]
        in_maps.append(
            {
                "xt": xt,
                "wq": np.ascontiguousarray(
                    wq_c.reshape(KT, P, NPAIR * P).transpose(1, 0, 2)
                ).astype(bf),
                "wkv": np.ascontiguousarray(
                    wkv_c.reshape(KT, P, P).transpose(1, 0, 2)
                ).astype(bf),
                "wo": np.ascontiguousarray(
                    wo_c.reshape(2, P, D).transpose(1, 0, 2)
                ).astype(bf),
                "cost": cost,
                "sint": sint,
                "bq": np.ascontiguousarray(bq_c, dtype=np.float32),
                "bkv": np.ascontiguousarray(bkv_c, dtype=np.float32),
            }
        )
    return in_maps


def run(in_maps, trace=False, **kw):
    nc = _get_nc()
    return run_bass_kernel_spmd(nc, in_maps, list(range(NCORES)), trace=trace, **kw)


def kernel(**inputs):
    in_maps = prepare_in_maps(**{k: inputs[k] for k in (
        "x", "freqs", "Wq", "bq", "Wk", "bk", "Wv", "bv", "Wo", "bo")})
    res = run(in_maps, trace=False)
    acc = np.zeros((B * S, D), np.float64)
    for r in res.results:
        acc += r["out"].astype(np.float64)
    out = acc.astype(np.float32) + np.asarray(inputs["bo"], np.float32)[None, :]
    return out.reshape(B, S, D)
